# revision 1
# baseline (speedup 1.0000x reference)
"""Trainium2 Bass kernel for nn_CiderFeatures (all-pairs Gaussian reduction).

y[i, c] = norms[c] * sum_j exp(-(a_j + b_ic) * ||x_i - x_j||^2) * f_j

Key structure exploited:
  b_i1 = b_i2 / 2,  b_i3 = 2 * b_i2   (exact, from the B/C coefficient table)
so with Ea = -a_j d^2 + lnf_j and Eb2 = -(b_i2/2) d^2:
  w1 = exp(Ea + Eb2)        (channel c=1, ACT exp, fp32 accum -> y1)
  u  = exp(Eb2)             (ACT exp)
  w2 = w1 * u               (DVE ttr, accum -> y2)
  w3 = w2 * u * u           (DVE tt + ttr, accum -> y3)
Two ACT exp passes instead of three, with the c=2/c=3 channels derived by
cheap vector multiplies.

Work reduction (host-side, data-dependent static schedule):
  - Points are kd-sorted into 128 spatially tight i-tiles of 128 points.
  - For each i-tile only j-columns with max_i arg > THR contribute
    (Gaussians decay fast; ~6% of columns survive at THR=-15, with a
    rigorous bound on the dropped mass).  Surviving columns are gathered
    densely on the host into per-tile packed buffers, so the device only
    computes on live pairs.
  - SPMD constraint (one program, 8 cores): per-slot column counts are
    equalized across cores by padding with the next-best columns (which
    only *adds* accurate terms; no wasted garbage work).

The exp argument is evaluated as a bf16-split bilinear form (TensorE into
PSUM, fp32 accumulate), as in the classic split-matmul trick: each factor
is decomposed into bf16 levels and cross products up to a per-dim level
budget are stacked in the contraction dim.
"""

import numpy as np
import ml_dtypes
from math import pi

N_CORES = 8
IT = 128               # i-tile size (partition dim)
CMAX = 1024            # max columns per chunk (2 PSUM banks fp32)
MM_N = 512             # matmul free-size chunk (1 PSUM bank)
THR = -6.75            # keep (i-tile, j) if max_i arg_c > THR
                       # (measured truncated mass: ~4e-3 rel vs the 2e-2
                       #  correctness gate; bf16 chain noise ~3e-4)
LNF_FLOOR = -100.0
F32 = np.float64       # host math dtype

_NC_CACHE = {}
_LAST = {}


# ---------------------------------------------------------------------------
# Host math
# ---------------------------------------------------------------------------

def _derived(rho, gamma, coords, weights):
    A, D = 2.0, 2.0
    B2, C2 = A, (6.0 * pi ** 2) ** (2.0 / 3.0) * (6.0 * A / (160.0 * pi))
    Bs = np.array([D / A * B2, B2 / 2.0, B2, 2.0 * B2])
    Cs = np.array([D / A * C2, C2 / 2.0, C2, 2.0 * C2])
    norms = ((Bs[0] + Bs[1:]) / 2.0) ** 1.5          # (3,)

    rho_ = rho + 1e-8
    t_w = gamma / (8.0 * rho_)
    t_tf = 0.3 * (3.0 * pi ** 2) ** (2.0 / 3.0) * rho_ ** (5.0 / 3.0)
    x = t_w / t_tf
    scale = pi * (rho_ / 2.0) ** (2.0 / 3.0)
    a = scale * (Bs[0] + Cs[0] * x)                  # Vj exponent
    b2 = scale * (Bs[2] + Cs[2] * x)                 # middle Vi exponent
    f = weights * rho
    lnf = np.maximum(np.log(np.maximum(f, 1e-300)), LNF_FLOOR)
    r = np.sum(coords * coords, axis=1)
    return a, b2, f, lnf, r, norms


def _kd_order(coords, leaf):
    """Recursive median split -> spatially tight tiles of `leaf` points."""
    n = coords.shape[0]
    out = []

    def rec(idx):
        if len(idx) <= leaf:
            out.append(idx)
            return
        c = coords[idx]
        dim = int(np.argmax(c.max(0) - c.min(0)))
        k = len(idx) // 2
        part = np.argpartition(c[:, dim], k)
        rec(idx[part[:k]])
        rec(idx[part[k:]])

    rec(np.arange(n))
    return np.concatenate(out)


def _survivors(coords_s, a_s, b2_s, lnf_s, n_it):
    """Per i-tile: exact per-column max-arg for each channel (t=1/2,1,2).

    Box-bound prefilter, exact refinement on the prefiltered set.
    Returns maxargs[3, n_it, N] (filled with -inf where prefiltered out,
    the box upper bound where refined out -- still usable for ranking
    padding candidates)."""
    N = coords_s.shape[0]
    tvals = (0.5, 1.0, 2.0)
    maxargs = np.full((3, n_it, N), -np.inf, dtype=np.float64)
    for ib in range(n_it):
        xi = coords_s[ib * IT:(ib + 1) * IT]
        lo, hi = xi.min(0), xi.max(0)
        dd = np.maximum(np.maximum(lo[None, :] - coords_s,
                                   coords_s - hi[None, :]), 0.0)
        d2min = np.sum(dd * dd, axis=1)
        bmin = b2_s[ib * IT:(ib + 1) * IT].min()
        ub0 = lnf_s - (a_s + tvals[0] * bmin) * d2min
        cand = np.where(ub0 > THR - 1.0)[0]
        # exact args on the candidate set
        d2 = np.sum((xi[:, None, :] - coords_s[cand][None, :, :]) ** 2, axis=2)
        for ci, t in enumerate(tvals):
            arg = -(a_s[cand][None, :]
                    + t * b2_s[ib * IT:(ib + 1) * IT, None]) * d2 \
                + lnf_s[cand][None, :]
            maxargs[ci, ib, cand] = arg.max(0)
        # keep a (slightly pessimistic) ranking value for non-candidates
        rest = np.where(ub0 <= THR - 1.0)[0]
        maxargs[0, ib, rest] = ub0[rest] - 1e3  # strictly below all candidates
        maxargs[1, ib, rest] = ub0[rest] - 1e3
        maxargs[2, ib, rest] = ub0[rest] - 1e3
    return maxargs


def _rup(n, m=4):
    return ((n + m - 1) // m) * m


def _make_schedule(maxargs, n_it):
    """Column lists per i-tile + SPMD-equalized slot structure.

    Per i-tile the packed column order is [S2 | S1\\S2 | S0\\S1] where
    Sc = columns alive for channel c.  Cores get i-tiles via greedy load
    balance; slot k of every core holds that core's k-th largest tile,
    padded with next-best columns to the global per-slot (n0, n1, n2).

    Returns (assign [n_it] -> core, slot_of [n_it], cols [n_it] -> packed
    j-array, slot_sizes [SLOTS] -> (n0, n1, n2), SLOTS)."""
    alive0 = maxargs[0] > THR
    alive1 = maxargs[1] > THR
    alive2 = maxargs[2] > THR
    n0 = alive0.sum(1)

    # Core assignment: snake-deal by n0, then local-search swaps to
    # minimize the total padded columns sum_k max_core(section sizes).
    SLOTS = n_it // N_CORES
    sec2c = alive2.sum(1)
    sec1c = (alive1 & ~alive2).sum(1)
    sec0c = (alive0 & ~alive1).sum(1)
    srt = np.argsort(-n0)
    core_tiles = [[] for _ in range(N_CORES)]
    for rk, ib in enumerate(srt):
        row, col = rk // N_CORES, rk % N_CORES
        c = col if row % 2 == 0 else N_CORES - 1 - col
        core_tiles[c].append(int(ib))

    def padded_total(cts):
        tot = 0
        for k in range(SLOTS):
            s2 = max(sec2c[cts[c][k]] for c in range(N_CORES))
            s1 = max(sec1c[cts[c][k]] for c in range(N_CORES))
            s0 = max(sec0c[cts[c][k]] for c in range(N_CORES))
            # weight: s2-cols do full chain, s1 adds u/w2, s0 only w1
            tot += 2 * (s2 + s1 + s0) + (s2 + s1) + s2
            tot += 700 * ((s2 + s1 + s0 + 1023) // 1024)  # per-chunk overhead
        return tot

    rng = np.random.default_rng(0)
    cur = padded_total(core_tiles)
    for _ in range(40000):
        c1, c2 = rng.integers(0, N_CORES, 2)
        if c1 == c2:
            continue
        k1, k2 = rng.integers(0, SLOTS, 2)
        core_tiles[c1][k1], core_tiles[c2][k2] = \
            core_tiles[c2][k2], core_tiles[c1][k1]
        new = padded_total(core_tiles)
        if new <= cur:
            cur = new
        else:
            core_tiles[c1][k1], core_tiles[c2][k2] = \
                core_tiles[c2][k2], core_tiles[c1][k1]

    # jointly permute slot indices: a small slot first (fast pipeline
    # fill), then descending, smallest last (short drain tail)
    gmax = np.asarray([max(n0[core_tiles[c][k]] for c in range(N_CORES))
                       for k in range(SLOTS)])
    g2 = np.asarray([max(sec2c[core_tiles[c][k]] for c in range(N_CORES))
                     for k in range(SLOTS)])
    if SLOTS >= 3:
        last = int(np.argmin(g2))          # shortest post-ACT chain at the tail
        rest = [k for k in np.argsort(-gmax) if k != last]
        perm = [rest[-1], rest[-2], rest[-3], rest[-4]] + rest[:-4] + [last]
    else:
        perm = list(np.argsort(-gmax))
    core_tiles = [[cts[k] for k in perm] for cts in core_tiles]

    slot_of = np.zeros(n_it, int)
    assign = np.zeros(n_it, int)
    core_slots = []
    for c in range(N_CORES):
        tiles = np.array(core_tiles[c], int)
        core_slots.append(tiles)
        for k, ib in enumerate(tiles):
            slot_of[ib] = k
            assign[ib] = c

    # global slot sizes: per-SECTION maxima so every core's class lists fit
    slot_sizes = []
    for k in range(SLOTS):
        sec2 = max(int(alive2[core_slots[c][k]].sum())
                   for c in range(N_CORES))
        sec1 = max(int((alive1[core_slots[c][k]]
                        & ~alive2[core_slots[c][k]]).sum())
                   for c in range(N_CORES))
        sec0 = max(int((alive0[core_slots[c][k]]
                        & ~alive1[core_slots[c][k]]).sum())
                   for c in range(N_CORES))
        s2 = _rup(max(sec2, 4))
        s1 = _rup(s2 + sec1)
        s0 = _rup(s1 + sec0)
        slot_sizes.append((s0, s1, s2))

    # per-tile padded column lists
    cols = [None] * n_it
    for ib in range(n_it):
        s0, s1, s2 = slot_sizes[slot_of[ib]]
        a2 = np.where(alive2[ib])[0]
        a1 = np.where(alive1[ib] & ~alive2[ib])[0]
        a0 = np.where(alive0[ib] & ~alive1[ib])[0]
        used = np.zeros(maxargs.shape[2], bool)
        used[a2] = used[a1] = used[a0] = True

        def take(pool_rank, want, used):
            # best unused columns by channel-specific maxarg
            cand = np.argsort(-pool_rank)
            picked = []
            for j in cand:
                if len(picked) >= want:
                    break
                if not used[j]:
                    picked.append(j)
                    used[j] = True
            return np.array(picked, int)

        p2 = take(maxargs[2, ib], s2 - len(a2), used)
        sec2 = np.concatenate([a2, p2]) if len(p2) else a2
        p1 = take(maxargs[1, ib], (s1 - s2) - len(a1), used)
        sec1 = np.concatenate([a1, p1]) if len(p1) else a1
        p0 = take(maxargs[0, ib], (s0 - s1) - len(a0), used)
        sec0 = np.concatenate([a0, p0]) if len(p0) else a0
        cols[ib] = np.concatenate([sec2, sec1, sec0]).astype(np.int64)
        assert len(cols[ib]) == s0
    return assign, slot_of, core_slots, cols, slot_sizes, SLOTS


# ---------------------------------------------------------------------------
# bf16-split bilinear decomposition
# ---------------------------------------------------------------------------

def _bf16_levels(M, nlev=3):
    rem = np.asarray(M, np.float64).copy()
    outs = []
    for _ in range(nlev):
        h = np.asarray(rem, ml_dtypes.bfloat16).astype(np.float64)
        outs.append(h)
        rem = rem - h
    return outs


def _split_dims(dims):
    """dims: list of (V_i [n_i], U_j [n_j], max_level_sum).
    Returns (Vrows [K, n_i], Urows [K, n_j]) bf16-representable float32."""
    vrows, urows = [], []
    for V, U, msum in dims:
        Vl = _bf16_levels(V)
        Ul = _bf16_levels(U)
        nv = 1 if np.all(V == V.astype(ml_dtypes.bfloat16).astype(np.float64)) else 3
        nu = 1 if np.all(U == U.astype(ml_dtypes.bfloat16).astype(np.float64)) else 3
        for lv in range(min(nv, 3)):
            for lu in range(min(nu, 3)):
                if lv + lu > msum:
                    continue
                v, u = Vl[lv], Ul[lu]
                if not v.any() or not u.any():
                    continue
                vrows.append(v)
                urows.append(u)
    return (np.stack(vrows).astype(np.float32),
            np.stack(urows).astype(np.float32))


def _build_vu(a, b2, lnf, r, coords_s):
    """Ea-side and Eb2-side split factor matrices (global, sorted order).

    Ea  = -a_j (r_i + r_j - 2 x_i.x_j) + lnf_j
    Eb2 = -(b_i/2)(r_i + r_j - 2 x_i.x_j)
    """
    n = a.shape[0]
    ones = np.ones(n)
    rbar = float(r.mean())
    rc = r - rbar
    xyz = coords_s

    ea_dims = [
        (rc, -a, 2),                                   # -a_j rc_i
        (ones, -a * (r + rbar) + lnf, 2),              # pure-j remainder
    ]
    for d in range(3):
        ea_dims.append((2.0 * xyz[:, d], a * xyz[:, d], 3))
    eb_dims = [
        (-0.5 * b2 * (r + rbar), ones, 2),             # pure-i remainder
        (-0.5 * b2, rc, 3),                            # -(b/2) rc_j
    ]
    for d in range(3):
        eb_dims.append((b2 * xyz[:, d], xyz[:, d], 3))

    va, ua = _split_dims(ea_dims)
    vb, ub = _split_dims(eb_dims)
    return va, ua, vb, ub


# ---------------------------------------------------------------------------
# Device program
# ---------------------------------------------------------------------------

def _chunks_of(slot_sizes):
    """Static chunk list: (slot, q0, na, nb, nc2)."""
    chunks = []
    for k, (s0, s1, s2) in enumerate(slot_sizes):
        q0 = 0
        while q0 < s0:
            na = min(CMAX, s0 - q0)
            nb = min(max(s1 - q0, 0), na)
            nc2 = min(max(s2 - q0, 0), na)
            chunks.append((k, q0, na, nb, nc2))
            q0 += na
    return chunks


def _plan_modes(chunks):
    """Greedy per-chunk engine balance (Pool's software ALU is 4x slower
    per element and its big serial beads stall the DVE chain, so it is
    not used).  Per chunk: y3 reduction via DVE stt, or via DVE tt
    product + ACT Copy+accum when DVE is ahead of ACT."""
    ACTC, STT, TT = 0.8333, 1.0417, 0.5208
    actT = dveT = 0.0
    modes = []
    for (k, q0, na, nb, nc2) in chunks:
        actT += (na + nb) * ACTC + 680          # two exps + accum aux
        # y2: DVE stt, or DVE tt product + ACT Copy+accum
        mS = max(actT, dveT + nb * STT + 190)
        mA = max(actT + (nb * ACTC + 430) * 3.0, dveT + nb * TT + 190)
        y2_act = mA < mS and nb > 0
        if y2_act:
            actT += nb * ACTC + 430
            dveT += nb * TT + 190
        else:
            dveT += nb * STT + 190
        if nc2:
            dveT += nc2 * TT + 190              # tmp product
            mS = max(actT, dveT + nc2 * STT + 190)
            mA = max(actT + (nc2 * ACTC + 430) * 3.0,
                     dveT + nc2 * TT + 190)
            y3_act = mA < mS
            if y3_act:
                actT += nc2 * ACTC + 430
                dveT += nc2 * TT + 190
            else:
                dveT += nc2 * STT + 190
        else:
            y3_act = False
        modes.append((y2_act, y3_act))
    for i in range(max(0, len(modes) - 2), len(modes)):
        modes[i] = (False, False)
    return modes, (actT, dveT, 0.0)


def _build_nc(key):
    """key = (K_a, K_b, slot_sizes tuple)."""
    K_a, K_b, slot_sizes = key
    slot_sizes = list(slot_sizes)
    import concourse.bass as bass  # noqa: F401
    import concourse.tile as tile
    from concourse import bacc, mybir
    from concourse.alu_op_type import AluOpType

    SLOTS = len(slot_sizes)
    chunks = _chunks_of(slot_sizes)
    NCH = len(chunks)
    offs = np.cumsum([0] + [s[0] for s in slot_sizes])
    modes, _ = _plan_modes(chunks)

    nc = bacc.Bacc("TRN2", target_bir_lowering=False)
    ua_dram = nc.dram_tensor("ua", [K_a, int(offs[-1])], mybir.dt.bfloat16,
                             kind="ExternalInput")
    ub_dram = nc.dram_tensor("ub", [K_b, int(offs[-1])], mybir.dt.bfloat16,
                             kind="ExternalInput")
    va_dram = nc.dram_tensor("va", [K_a, SLOTS * IT], mybir.dt.bfloat16,
                             kind="ExternalInput")
    vb_dram = nc.dram_tensor("vb", [K_b, SLOTS * IT], mybir.dt.bfloat16,
                             kind="ExternalInput")
    y_dram = nc.dram_tensor("y", [IT, 3 * NCH], mybir.dt.float32,
                            kind="ExternalOutput")

    with tile.TileContext(nc) as tc:
        with (
            tc.tile_pool(name="singles", bufs=1) as singles,
            tc.tile_pool(name="psum", bufs=4, space="PSUM") as psum_pool,
            tc.tile_pool(name="wpool", bufs=4) as wpool,
        ):
            warm = singles.tile([128, 1], mybir.dt.float32)
            nc.vector.memset(warm[:], 0.0)
            # PE warm-up source data (dummy matmuls run inside chunk 0's
            # psum tile so no extra PSUM buffer is held).
            wmm = singles.tile([1, 512], mybir.dt.bfloat16)
            nc.vector.memset(wmm[:], 0.0)

            va_sb = singles.tile([K_a, SLOTS * IT], mybir.dt.bfloat16)
            vb_sb = singles.tile([K_b, SLOTS * IT], mybir.dt.bfloat16)
            # U buffers: single tiles, loaded in a few big range-DMAs so
            # the first chunks can start while the tail streams in; ua
            # goes through the ACT hwdge queue to halve queue serialization.
            TOT = int(offs[-1])
            ua_sb = singles.tile([K_a, TOT], mybir.dt.bfloat16)
            ub_sb = singles.tile([K_b, TOT], mybir.dt.bfloat16)
            cuts = sorted(set(int(offs[min(k, SLOTS)])
                              for k in (4, 5, 7)) | {0, TOT})
            # critical path to the first chunk: ub[slot0] (sync queue) and
            # vb (ACT queue) land in parallel; the ACT queue issues ONLY
            # the two small V loads (each dma_start costs ~0.7us of ACT
            # sequencer time ahead of the first exp).
            nc.scalar.dma_start(vb_sb[:], vb_dram[:])
            nc.scalar.dma_start(va_sb[:], va_dram[:])
            for lo, hi in zip(cuts[:-1], cuts[1:]):
                if hi > lo:
                    nc.sync.dma_start(ub_sb[:, lo:hi], ub_dram[:, lo:hi])
                    nc.sync.dma_start(ua_sb[:, lo:hi], ua_dram[:, lo:hi])
            # ACT exp-table warm AFTER the DMA issues: the implicit table
            # load (~1.3us) otherwise delays the vb/va issue on this queue.
            nc.scalar.activation(out=warm[:], in_=warm[:],
                                 func=mybir.ActivationFunctionType.Exp)

            parts = singles.tile([IT, 3 * NCH], mybir.dt.float32)
            nc.vector.memset(parts[:], 0.0)

            # 3-stage software-pipelined emission:
            #   A1(i): Eb/2 matmuls + u exp        (PE then ACT)
            #   A2(i): Ea accumulate + w1 exp + y2 (PE, ACT, DVE)
            #   B(i):  tmp product + y3            (DVE [+ACT])
            # Emitting A1(i+1) before A2(i) keeps ACT fed (u of the next
            # chunk is ready while PE accumulates Ea of the current one).
            state = [None] * NCH

            def emit_A1(ci):
                k, q0, na, nb, nc2 = chunks[ci]
                off = int(offs[k]) + q0
                lhs_b = vb_sb[:, k * IT:(k + 1) * IT]
                pt = psum_pool.tile([128, CMAX], mybir.dt.float32, tag="ps",
                                    name=f"pt{ci}")
                if ci == 0:
                    # p-state warm-up during the input-DMA window
                    for _ in range(10):
                        nc.tensor.matmul(pt[:1, :256], wmm[:, :1],
                                         wmm[:, :256], start=True, stop=True)
                for q in range(0, na, MM_N):
                    e = min(q + MM_N, na)
                    nc.tensor.matmul(pt[:, q:e], lhs_b,
                                     ub_sb[:, off + q:off + e],
                                     start=True, stop=True)
                u_t = wpool.tile([128, CMAX], mybir.dt.bfloat16, tag="u",
                                 name=f"u{ci}", bufs=6)
                if nb:
                    nc.scalar.activation(out=u_t[:, :nb], in_=pt[:, :nb],
                                         func=mybir.ActivationFunctionType.Exp)
                state[ci] = (pt, u_t, None, None)

            def emit_A2(ci):
                k, q0, na, nb, nc2 = chunks[ci]
                off = int(offs[k]) + q0
                lhs_a = va_sb[:, k * IT:(k + 1) * IT]
                pt, u_t, _, _ = state[ci]
                for q in range(0, na, MM_N):
                    e = min(q + MM_N, na)
                    nc.tensor.matmul(pt[:, q:e], lhs_a,
                                     ua_sb[:, off + q:off + e],
                                     start=False, stop=True,
                                     skip_group_check=True)
                w1_t = wpool.tile([128, CMAX], mybir.dt.bfloat16, tag="w1",
                                  name=f"w1{ci}")
                nc.scalar.activation(out=w1_t[:, :na], in_=pt[:, :na],
                                     func=mybir.ActivationFunctionType.Exp,
                                     accum_out=parts[:, 3 * ci:3 * ci + 1])
                w2_t = None
                if nb and ci != NCH - 1:
                    y2_act, _ = modes[ci]
                    w2_t = wpool.tile([128, CMAX], mybir.dt.bfloat16,
                                      tag="w2", name=f"w2{ci}")
                    if y2_act:
                        nc.vector.tensor_tensor(
                            w2_t[:, :nb], w1_t[:, :nb], u_t[:, :nb],
                            AluOpType.mult)
                    else:
                        nc.vector.scalar_tensor_tensor(
                            w2_t[:, :nb], w1_t[:, :nb], 1.0, u_t[:, :nb],
                            AluOpType.mult, AluOpType.mult,
                            accum_out=parts[:, 3 * ci + 1:3 * ci + 2])
                state[ci] = (pt, u_t, w1_t, w2_t)

            def emit_B(ci):
                k, q0, na, nb, nc2 = chunks[ci]
                y2_act, y3_act = modes[ci]
                _, u_t, _, w2_t = state[ci]
                if y2_act and nb:
                    w2c_t = wpool.tile([128, CMAX], mybir.dt.bfloat16,
                                       tag="w3c", name=f"w2c{ci}", bufs=2)
                    nc.scalar.activation(
                        out=w2c_t[:, :nb], in_=w2_t[:, :nb],
                        func=mybir.ActivationFunctionType.Copy,
                        accum_out=parts[:, 3 * ci + 1:3 * ci + 2])
                if not nc2:
                    return
                tmp_pool = False
                tmp_t = wpool.tile([128, CMAX], mybir.dt.bfloat16, tag="tmp",
                                   name=f"tmp{ci}")
                eng = nc.gpsimd if tmp_pool else nc.vector
                eng.tensor_tensor(tmp_t[:, :nc2], w2_t[:, :nc2],
                                  u_t[:, :nc2], AluOpType.mult)
                w3_t = wpool.tile([128, CMAX], mybir.dt.bfloat16, tag="w3",
                                  name=f"w3{ci}")
                if y3_act:
                    nc.vector.tensor_tensor(w3_t[:, :nc2], tmp_t[:, :nc2],
                                            u_t[:, :nc2], AluOpType.mult)
                    w3c_t = wpool.tile([128, CMAX], mybir.dt.bfloat16,
                                       tag="w3c", name=f"w3c{ci}", bufs=2)
                    nc.scalar.activation(
                        out=w3c_t[:, :nc2], in_=w3_t[:, :nc2],
                        func=mybir.ActivationFunctionType.Copy,
                        accum_out=parts[:, 3 * ci + 2:3 * ci + 3])
                else:
                    nc.vector.scalar_tensor_tensor(
                        w3_t[:, :nc2], tmp_t[:, :nc2], 1.0, u_t[:, :nc2],
                        AluOpType.mult, AluOpType.mult,
                        accum_out=parts[:, 3 * ci + 2:3 * ci + 3])

            def emit_tail3(ci):
                k, q0, na, nb, nc2 = chunks[ci]
                off = int(offs[k]) + q0
                lhs_b = vb_sb[:, k * IT:(k + 1) * IT]
                pt, u_t, w1_t, w2_t = state[ci]
                for q in range(0, nb, MM_N):
                    e = min(q + MM_N, nb)
                    nc.tensor.matmul(pt[:, q:e], lhs_b,
                                     ub_sb[:, off + q:off + e],
                                     start=False, stop=True,
                                     skip_group_check=True)
                w2x = wpool.tile([128, CMAX], mybir.dt.bfloat16, tag="w2",
                                 name=f"w2x{ci}")
                nc.scalar.activation(out=w2x[:, :nb], in_=pt[:, :nb],
                                     func=mybir.ActivationFunctionType.Exp,
                                     accum_out=parts[:, 3 * ci + 1:3 * ci + 2])
                for _ in range(2):
                    for q in range(0, nc2, MM_N):
                        e = min(q + MM_N, nc2)
                        nc.tensor.matmul(pt[:, q:e], lhs_b,
                                         ub_sb[:, off + q:off + e],
                                         start=False, stop=True,
                                         skip_group_check=True)
                w3x = wpool.tile([128, CMAX], mybir.dt.bfloat16, tag="w3",
                                 name=f"w3x{ci}")
                nc.scalar.activation(out=w3x[:, :nc2], in_=pt[:, :nc2],
                                     func=mybir.ActivationFunctionType.Exp,
                                     accum_out=parts[:, 3 * ci + 2:3 * ci + 3])

            for ci in range(NCH + 2):
                if ci < NCH:
                    emit_A1(ci)
                if 1 <= ci <= NCH:
                    if ci - 1 == NCH - 1:
                        emit_A2(ci - 1)
                        emit_tail3(ci - 1)
                    else:
                        emit_A2(ci - 1)
                if ci >= 2 and ci - 2 != NCH - 1:
                    emit_B(ci - 2)
            nc.sync.dma_start(y_dram[:], parts[:])
    nc.finalize()
    return nc


# ---------------------------------------------------------------------------
# Driver
# ---------------------------------------------------------------------------

def _prep(rho, gamma, coords, weights):
    rho = np.asarray(rho, F32)
    gamma = np.asarray(gamma, F32)
    coords = np.asarray(coords, F32)
    weights = np.asarray(weights, F32)
    n = rho.shape[0]
    n_it = n // IT

    a, b2, f, lnf, r, norms = _derived(rho, gamma, coords, weights)
    order = _kd_order(coords, IT)
    cs, as_, b2s, lnfs, rs = (coords[order], a[order], b2[order],
                              lnf[order], r[order])
    maxargs = _survivors(cs, as_, b2s, lnfs, n_it)
    assign, slot_of, core_slots, cols, slot_sizes, SLOTS = \
        _make_schedule(maxargs, n_it)
    va, ua, vb, ub = _build_vu(as_, b2s, lnfs, rs, cs)
    K_a, K_b = va.shape[0], vb.shape[0]

    in_maps = []
    for c in range(N_CORES):
        tiles = core_slots[c]
        uac = np.concatenate([ua[:, cols[ib]] for ib in tiles], axis=1)
        ubc = np.concatenate([ub[:, cols[ib]] for ib in tiles], axis=1)
        vac = np.concatenate(
            [va[:, ib * IT:(ib + 1) * IT] for ib in tiles], axis=1)
        vbc = np.concatenate(
            [vb[:, ib * IT:(ib + 1) * IT] for ib in tiles], axis=1)
        in_maps.append({
            "ua": np.ascontiguousarray(uac.astype(ml_dtypes.bfloat16)),
            "ub": np.ascontiguousarray(ubc.astype(ml_dtypes.bfloat16)),
            "va": np.ascontiguousarray(vac.astype(ml_dtypes.bfloat16)),
            "vb": np.ascontiguousarray(vbc.astype(ml_dtypes.bfloat16)),
        })
    key = (K_a, K_b, tuple(slot_sizes))
    meta = dict(order=order, core_slots=core_slots, norms=norms,
                slot_sizes=slot_sizes, n=n)
    return key, in_maps, meta


def _assemble(results, meta):
    n = meta["n"]
    norms = meta["norms"]
    chunks = _chunks_of(meta["slot_sizes"])
    order = meta["order"]
    y = np.zeros((n, 3), np.float64)
    for c in range(N_CORES):
        parts = np.asarray(results[c]["y"], np.float64)  # [IT, 3*NCH]
        acc = np.zeros((len(meta["core_slots"][c]), IT, 3))
        for ci, (k, q0, na, nb, nc2) in enumerate(chunks):
            acc[k, :, :] += parts[:, 3 * ci:3 * ci + 3]
        for k, ib in enumerate(meta["core_slots"][c]):
            rows = order[ib * IT:(ib + 1) * IT]
            y[rows, :] = acc[k] * norms[None, :]
    return y.astype(np.float32)


def kernel_run(rho, gamma, coords, weights, **spmd_kwargs):
    from concourse.bass_utils import run_bass_kernel_spmd

    key, in_maps, meta = _prep(rho, gamma, coords, weights)
    if key not in _NC_CACHE:
        _NC_CACHE[key] = _build_nc(key)
    _LAST["key"] = key
    _LAST["meta"] = meta
    _LAST["in_maps"] = in_maps
    res = run_bass_kernel_spmd(_NC_CACHE[key], in_maps,
                               core_ids=list(range(N_CORES)), **spmd_kwargs)
    return _assemble(res.results, meta), res


def kernel(rho, gamma, coords, weights):
    y, _ = kernel_run(rho, gamma, coords, weights)
    return y



# revision 46
# speedup vs baseline: 1.2814x; 1.2814x over previous
"""Trainium2 Bass kernel for nn_CiderFeatures (all-pairs Gaussian reduction).

y[i, c] = norms[c] * sum_j exp(-(a_j + b_ic) * ||x_i - x_j||^2) * f_j

Structure (v2 — grouped single-section pipeline):
  b_i1 = b_i2 / 2,  b_i3 = 2 * b_i2  (exact), so with Eb2 = -(b_i2/2) d^2
  and Ea = -a_j d^2 + lnf_j:
    u  = exp(Eb2)                  (ACT exp, per group)
    w1 = exp(Ea + Eb2)             (ACT exp, per group)
    w2 = w1 * u,  u2 = u * u,  w3 = w2 * u2   (DVE/Pool products, bf16 2x)
    y_c = sum_j w_c  via DVE tensor_scalar+accum (4x mode, 0.26 ns/col)

Work reduction (host-side, data-dependent static schedule):
  - Points kd-sorted into 128 spatially tight i-tiles of 128 points.
  - Single survivor section per i-tile: keep column j if ANY channel's
    max_i arg_c > THR[c]; all three channels are evaluated on every kept
    column (the marginal per-channel sections are tiny at the tuned
    thresholds, and extra terms only reduce error).
  - SPMD: per-slot column counts equalized across cores by padding with
    next-best columns (adds accurate terms only).
  - Slots are bin-packed into PSUM groups (<= GROUP cols, 4 banks) so the
    two exps per group are emitted as single merged ACT instructions,
    amortizing the ~185/372 ns per-instruction ACT overheads.

The exp argument is evaluated as a bf16-split bilinear form (TensorE into
PSUM, fp32 accumulate): each factor decomposed into bf16 levels, cross
products up to a level budget stacked in the contraction dim.
"""

import numpy as np
import ml_dtypes
from math import pi

N_CORES = 8
IT = 128               # i-tile size (partition dim)
GROUP = 1024           # psum group tile cols (2 fp32 banks, 4 in flight)
MM_N = 512             # matmul free-size chunk (1 PSUM bank)
THR = (-5.2, -5.6, -6.1)   # per-channel: keep (i-tile, j) if
                       # max_i arg_c > THR[c]
THRPRE = min(THR) - 0.75   # box-bound prefilter cut
LNF_FLOOR = -100.0
F32 = np.float64       # host math dtype

_NC_CACHE = {}
_LAST = {}


# ---------------------------------------------------------------------------
# Host math
# ---------------------------------------------------------------------------

def _derived(rho, gamma, coords, weights):
    A, D = 2.0, 2.0
    B2, C2 = A, (6.0 * pi ** 2) ** (2.0 / 3.0) * (6.0 * A / (160.0 * pi))
    Bs = np.array([D / A * B2, B2 / 2.0, B2, 2.0 * B2])
    Cs = np.array([D / A * C2, C2 / 2.0, C2, 2.0 * C2])
    norms = ((Bs[0] + Bs[1:]) / 2.0) ** 1.5          # (3,)

    rho_ = rho + 1e-8
    t_w = gamma / (8.0 * rho_)
    t_tf = 0.3 * (3.0 * pi ** 2) ** (2.0 / 3.0) * rho_ ** (5.0 / 3.0)
    x = t_w / t_tf
    scale = pi * (rho_ / 2.0) ** (2.0 / 3.0)
    a = scale * (Bs[0] + Cs[0] * x)                  # Vj exponent
    b2 = scale * (Bs[2] + Cs[2] * x)                 # middle Vi exponent
    f = weights * rho
    lnf = np.maximum(np.log(np.maximum(f, 1e-300)), LNF_FLOOR)
    r = np.sum(coords * coords, axis=1)
    return a, b2, f, lnf, r, norms


def _kd_order(coords, leaf):
    """Recursive median split -> spatially tight tiles of `leaf` points."""
    n = coords.shape[0]
    out = []

    def rec(idx):
        if len(idx) <= leaf:
            out.append(idx)
            return
        c = coords[idx]
        dim = int(np.argmax(c.max(0) - c.min(0)))
        k = len(idx) // 2
        part = np.argpartition(c[:, dim], k)
        rec(idx[part[:k]])
        rec(idx[part[k:]])

    rec(np.arange(n))
    return np.concatenate(out)


def _survivors(coords_s, a_s, b2_s, lnf_s, n_it):
    """Per i-tile: exact per-column max-arg for each channel (t=1/2,1,2).

    Box-bound prefilter, exact refinement on the prefiltered set.
    Returns maxargs[3, n_it, N]."""
    N = coords_s.shape[0]
    tvals = (0.5, 1.0, 2.0)
    maxargs = np.full((3, n_it, N), -np.inf, dtype=np.float64)
    for ib in range(n_it):
        xi = coords_s[ib * IT:(ib + 1) * IT]
        lo, hi = xi.min(0), xi.max(0)
        dd = np.maximum(np.maximum(lo[None, :] - coords_s,
                                   coords_s - hi[None, :]), 0.0)
        d2min = np.sum(dd * dd, axis=1)
        bmin = b2_s[ib * IT:(ib + 1) * IT].min()
        ub0 = lnf_s - (a_s + tvals[0] * bmin) * d2min
        cand = np.where(ub0 > THRPRE)[0]
        d2 = np.sum((xi[:, None, :] - coords_s[cand][None, :, :]) ** 2, axis=2)
        for ci, t in enumerate(tvals):
            arg = -(a_s[cand][None, :]
                    + t * b2_s[ib * IT:(ib + 1) * IT, None]) * d2 \
                + lnf_s[cand][None, :]
            maxargs[ci, ib, cand] = arg.max(0)
        rest = np.where(ub0 <= THRPRE)[0]
        maxargs[0, ib, rest] = ub0[rest] - 1e3
        maxargs[1, ib, rest] = ub0[rest] - 1e3
        maxargs[2, ib, rest] = ub0[rest] - 1e3
    return maxargs


def _rup(n, m=4):
    return ((n + m - 1) // m) * m


def _make_schedule(maxargs, n_it):
    """Single-section schedule: per-tile packed column lists + core
    assignment + SPMD-equalized slot sizes + psum group packing.

    Returns (assign, slot_of, core_slots, cols, slot_sizes, groups)."""
    alive = ((maxargs[0] > THR[0]) | (maxargs[1] > THR[1])
             | (maxargs[2] > THR[2]))
    nn = alive.sum(1)                                 # per-tile count

    SLOTS = n_it // N_CORES
    srt = np.argsort(-nn)
    core_tiles = [[] for _ in range(N_CORES)]
    for rk, ib in enumerate(srt):
        row, col = rk // N_CORES, rk % N_CORES
        c = col if row % 2 == 0 else N_CORES - 1 - col
        core_tiles[c].append(int(ib))

    def padded_total(cts):
        return sum(max(nn[cts[c][k]] for c in range(N_CORES))
                   for k in range(SLOTS))

    rng = np.random.default_rng(0)
    cur = padded_total(core_tiles)
    for _ in range(20000):
        c1, c2 = rng.integers(0, N_CORES, 2)
        if c1 == c2:
            continue
        k1, k2 = rng.integers(0, SLOTS, 2)
        core_tiles[c1][k1], core_tiles[c2][k2] = \
            core_tiles[c2][k2], core_tiles[c1][k1]
        new = padded_total(core_tiles)
        if new <= cur:
            cur = new
        else:
            core_tiles[c1][k1], core_tiles[c2][k2] = \
                core_tiles[c2][k2], core_tiles[c1][k1]

    # common slot sizes (max across cores, 4-aligned), sorted descending
    sizes = np.array([max(nn[core_tiles[c][k]] for c in range(N_CORES))
                      for k in range(SLOTS)])
    perm = list(np.argsort(-sizes))
    core_tiles = [[cts[k] for k in perm] for cts in core_tiles]
    slot_sizes = [_rup(max(int(sizes[k]), 4)) for k in perm]

    # split oversized slots into column-chunk units of <= GROUP cols;
    # units carry (slot, col_lo, col_len)
    units = []
    for k, s in enumerate(slot_sizes):
        npc = (s + GROUP - 1) // GROUP
        base = _rup(s // npc)
        lo = 0
        for j in range(npc):
            ln = min(base, s - lo) if j < npc - 1 else s - lo
            units.append((k, lo, ln))
            lo += ln
    units.sort(key=lambda u: -u[2])

    # bin-pack units into psum groups of <= GROUP cols: the two smallest
    # units open and close the pipeline; the rest is first-fit-descending
    # ordered small -> big -> small (pyramid) for fast ramp + short tail.
    bins = []
    for ui in range(len(units) - 2):
        ln = units[ui][2]
        placed = False
        for b in bins:
            if b[0] + ln <= GROUP:
                b[0] += ln
                b[1].append(ui)
                placed = True
                break
        if not placed:
            bins.append([ln, [ui]])
    bins.sort(key=lambda b: b[0])
    asc_odd = [b[1] for i, b in enumerate(bins) if i % 2 == 1]
    asc_even = [b[1] for i, b in enumerate(bins) if i % 2 == 0]
    groups = ([[len(units) - 2]] + asc_even + asc_odd[::-1]
              + [[len(units) - 1]])

    slot_of = np.zeros(n_it, int)
    assign = np.zeros(n_it, int)
    core_slots = []
    for c in range(N_CORES):
        tiles = np.array(core_tiles[c], int)
        core_slots.append(tiles)
        for k, ib in enumerate(tiles):
            slot_of[ib] = k
            assign[ib] = c

    # per-tile padded column lists (pad with next-best by best-channel
    # shifted rank; padding only adds accurate terms)
    rank = np.maximum(maxargs[0] - THR[0],
                      np.maximum(maxargs[1] - THR[1], maxargs[2] - THR[2]))
    cols = [None] * n_it
    for ib in range(n_it):
        s = slot_sizes[slot_of[ib]]
        base = np.where(alive[ib])[0]
        want = s - len(base)
        if want > 0:
            r = rank[ib].copy()
            r[base] = -np.inf
            pad = np.argpartition(-r, want)[:want]
            cl = np.concatenate([base, pad])
        else:
            cl = base
        cols[ib] = cl.astype(np.int64)
        assert len(cols[ib]) == s
    return assign, slot_of, core_slots, cols, slot_sizes, units, groups


# ---------------------------------------------------------------------------
# bf16-split bilinear decomposition
# ---------------------------------------------------------------------------

def _bf16_levels(M, nlev=3):
    rem = np.asarray(M, np.float64).copy()
    outs = []
    for _ in range(nlev):
        h = np.asarray(rem, ml_dtypes.bfloat16).astype(np.float64)
        outs.append(h)
        rem = rem - h
    return outs


def _split_dims(dims):
    vrows, urows = [], []
    for V, U, msum in dims:
        Vl = _bf16_levels(V)
        Ul = _bf16_levels(U)
        nv = 1 if np.all(V == V.astype(ml_dtypes.bfloat16).astype(np.float64)) else 3
        nu = 1 if np.all(U == U.astype(ml_dtypes.bfloat16).astype(np.float64)) else 3
        for lv in range(min(nv, 3)):
            for lu in range(min(nu, 3)):
                if lv + lu > msum:
                    continue
                v, u = Vl[lv], Ul[lu]
                if not v.any() or not u.any():
                    continue
                vrows.append(v)
                urows.append(u)
    return (np.stack(vrows).astype(np.float32),
            np.stack(urows).astype(np.float32))


def _build_vu(a, b2, lnf, r, coords_s):
    """Ea-side and Eb2-side split factor matrices (global, sorted order)."""
    n = a.shape[0]
    ones = np.ones(n)
    rbar = float(r.mean())
    rc = r - rbar
    xyz = coords_s

    ea_dims = [
        (rc, -a, 2),
        (ones, -a * (r + rbar) + lnf, 2),
    ]
    for d in range(3):
        ea_dims.append((2.0 * xyz[:, d], a * xyz[:, d], 3))
    eb_dims = [
        (-0.5 * b2 * (r + rbar), ones, 2),
        (-0.5 * b2, rc, 3),
    ]
    for d in range(3):
        eb_dims.append((b2 * xyz[:, d], xyz[:, d], 3))

    va, ua = _split_dims(ea_dims)
    vb, ub = _split_dims(eb_dims)
    return va, ua, vb, ub


# ---------------------------------------------------------------------------
# Device program
# ---------------------------------------------------------------------------

def _plan(units, groups):
    """Per-unit product-engine assignment, greedy 3-engine balance.

    Per group: u2 = u*u (merged) stays on DVE (it gates both engines'
    w3 products; a multi-us Pool bead there stalls everything).  Per
    unit: the w2/w3 products go to DVE or Pool (sigmas always DVE
    tensor_scalar 4x), picked to minimize the projected makespan with
    measured cost-model constants."""
    ACT_COL, DVE_TT, DVE_TS, POOL_TT = 0.8333, 0.5208, 0.2604, 1.9841
    A = 1283.0                                   # exp table load
    D = P = 0.0
    unit_src = {}
    for gi, g in enumerate(groups):
        gsz = sum(units[ui][2] for ui in g)
        A += 2 * (ACT_COL * gsz + 185.0)         # u exp + w1 exp
        D += DVE_TT * gsz + 60.0                 # u2
        for ui in sorted(g, key=lambda ui: -units[ui][2]):
            s = units[ui][2]
            D += 3 * (DVE_TS * s + 60.0)         # sigmas (always DVE)
            # products w2/w3: DVE tt at 2x, or Pool tensor_tensor (the
            # only ALU op codegen supports on Pool).  The last two groups
            # stay off Pool: its late beads would gate the final DMA.
            cD = 2 * (DVE_TT * s + 60.0)
            cP = 2 * (POOL_TT * s + 95.0)
            if gi >= len(groups) - 2 or D + cD <= P + cP:
                D += cD
                unit_src[ui] = "dve"
            else:
                P += cP
                unit_src[ui] = "pool"
    return unit_src, (A, D, P)


def _build_nc(key):
    """key = (K, K_b, units tuple, groups tuple-of-tuples)."""
    K, K_b, units, groups = key
    units = [tuple(u) for u in units]
    groups = [list(g) for g in groups]
    import concourse.tile as tile
    from concourse import bacc, mybir
    from concourse.alu_op_type import AluOpType

    NU = len(units)
    G = len(groups)
    TOT = sum(u[2] for u in units)
    unit_src, _ = _plan(units, groups)

    # ua/ub and va/vb column order: groups in order, units within group
    # in order (so per-group DMA pieces are contiguous)
    col_off = {}
    pos = {}
    off = 0
    p = 0
    for g in groups:
        for ui in g:
            col_off[ui] = off
            off += units[ui][2]
            pos[ui] = p
            p += 1
    grp_off = {}
    grp_sz = {}
    for gi, g in enumerate(groups):
        grp_off[gi] = col_off[g[0]]
        grp_sz[gi] = sum(units[ui][2] for ui in g)

    nc = bacc.Bacc("TRN2", target_bir_lowering=False)
    uab_dram = nc.dram_tensor("uab", [K, TOT], mybir.dt.bfloat16,
                              kind="ExternalInput")
    vab_dram = nc.dram_tensor("vab", [K, NU * IT], mybir.dt.bfloat16,
                              kind="ExternalInput")
    y_dram = nc.dram_tensor("y", [IT, 3 * NU], mybir.dt.float32,
                            kind="ExternalOutput")

    with tile.TileContext(nc) as tc:
        with (
            tc.tile_pool(name="singles", bufs=1) as singles,
            tc.tile_pool(name="psum", bufs=2, space="PSUM") as psum_pool,
            tc.tile_pool(name="wpool", bufs=3) as wpool,
        ):
            warm = singles.tile([128, 1], mybir.dt.float32)
            nc.vector.memset(warm[:], 0.0)

            vab_sb = singles.tile([K, NU * IT], mybir.dt.bfloat16)
            uab_sb = singles.tile([K, TOT], mybir.dt.bfloat16)

            # DMA pieces issued in consumption order on the sync hwdge
            # queue; the big ub tail goes on the pool swdge queue as a
            # second parallel stream.  vrng/urng give each group's
            # contiguous column ranges (both packings are group-ordered).
            def vrng(glo, ghi):
                lo = pos[groups[glo][0]] * IT
                hi = (pos[groups[ghi][-1]] + 1) * IT
                return lo, hi

            def urng(glo, ghi):
                return grp_off[glo], grp_off[ghi] + grp_sz[ghi]

            def dma(q, sb, dram, rng):
                lo, hi = rng
                q.dma_start(sb[:, lo:hi], dram[:, lo:hi])

            # first two groups (both small) merged into single pieces in
            # strict need order, then thirds for the rest; the u tail
            # rides the pool swdge queue as a second parallel stream
            dma(nc.sync, vab_sb, vab_dram, vrng(0, 1))
            dma(nc.sync, uab_sb, uab_dram, urng(0, 1))
            if G > 2:
                cuts = [2 + (G - 2) // 3, 2 + (2 * (G - 2)) // 3, G]
                lo = 2
                for hi in cuts:
                    if hi <= lo:
                        continue
                    dma(nc.sync, vab_sb, vab_dram, vrng(lo, hi - 1))
                    dma(nc.gpsimd, uab_sb, uab_dram, urng(lo, hi - 1))
                    lo = hi
            nc.scalar.activation(out=warm[:], in_=warm[:],
                                 func=mybir.ActivationFunctionType.Exp)

            parts = singles.tile([IT, 3 * NU], mybir.dt.float32)
            nc.vector.memset(parts[:], 0.0)

            state = [None] * G

            def mm_pieces(lo, sz):
                """Split [lo, lo+sz) at MM_N grid (psum bank) boundaries."""
                out = []
                q = lo
                while q < lo + sz:
                    e = min((q // MM_N + 1) * MM_N, lo + sz)
                    out.append((q, e))
                    q = e
                return out

            def emit_mm(gi, pt, kk):
                """start=True matmuls with the first kk stacked rows."""
                for ui in groups[gi]:
                    lhs = vab_sb[:kk, pos[ui] * IT:(pos[ui] + 1) * IT]
                    o = col_off[ui] - grp_off[gi]
                    for q, e in mm_pieces(o, units[ui][2]):
                        nc.tensor.matmul(
                            pt[:, q:e], lhs,
                            uab_sb[:kk, grp_off[gi] + q:grp_off[gi] + e],
                            start=True, stop=True)

            def emit_A(gi):
                """Eb2 matmuls + u exp."""
                gsz = grp_sz[gi]
                pt = psum_pool.tile([128, GROUP], mybir.dt.float32,
                                    tag="psu", name=f"ptu{gi}")
                emit_mm(gi, pt, K_b)
                u_t = wpool.tile([128, GROUP], mybir.dt.bfloat16, tag="u",
                                 name=f"u{gi}")
                nc.scalar.activation(out=u_t[:, :gsz], in_=pt[:, :gsz],
                                     func=mybir.ActivationFunctionType.Exp)
                state[gi] = (pt, u_t, None, None)

            def rng(gi, ui):
                o = col_off[ui] - grp_off[gi]
                return o, o + units[ui][2]

            def emit_B(gi):
                """Ea+Eb2 matmuls + u2 product + w1 exp + pool products."""
                gsz = grp_sz[gi]
                _, u_t = state[gi][:2]
                pt = psum_pool.tile([128, GROUP], mybir.dt.float32,
                                    tag="psw", name=f"ptw{gi}")
                emit_mm(gi, pt, K)
                u2_t = wpool.tile([128, GROUP], mybir.dt.bfloat16, tag="u2",
                                  name=f"u2{gi}")
                nc.vector.tensor_tensor(u2_t[:, :gsz], u_t[:, :gsz],
                                        u_t[:, :gsz], AluOpType.mult)
                w1_t = wpool.tile([128, GROUP], mybir.dt.bfloat16, tag="w1",
                                  name=f"w1{gi}")
                single = len(groups[gi]) == 1
                acc = {}
                if single:
                    p0 = pos[groups[gi][0]]
                    acc = dict(accum_out=parts[:, 3 * p0:3 * p0 + 1])
                nc.scalar.activation(out=w1_t[:, :gsz], in_=pt[:, :gsz],
                                     func=mybir.ActivationFunctionType.Exp,
                                     **acc)
                # pool-slot products right behind w1 so Pool gets a full
                # pipeline period before their sigmas are needed
                w2_t = wpool.tile([128, GROUP], mybir.dt.bfloat16, tag="w2",
                                  name=f"w2{gi}")
                w3_t = wpool.tile([128, GROUP], mybir.dt.bfloat16, tag="w3",
                                  name=f"w3{gi}")
                for ui in groups[gi]:
                    if unit_src[ui] != "pool":
                        continue
                    o, e = rng(gi, ui)
                    nc.gpsimd.tensor_tensor(w2_t[:, o:e], w1_t[:, o:e],
                                            u_t[:, o:e], AluOpType.mult)
                    nc.gpsimd.tensor_tensor(w3_t[:, o:e], w2_t[:, o:e],
                                            u2_t[:, o:e], AluOpType.mult)
                state[gi] = (pt, u_t, u2_t, w1_t, w2_t, w3_t)

            def sig(gi, ui, ci, w, scr):
                o, e = rng(gi, ui)
                p = pos[ui]
                nc.vector.tensor_scalar(
                    scr[:, o:e], w[:, o:e], 1.0, 0.0, AluOpType.mult,
                    AluOpType.add,
                    accum_out=parts[:, 3 * p + ci:3 * p + ci + 1])

            def emit_C(gi):
                """DVE products + dve-slot sigmas."""
                _, u_t, u2_t, w1_t, w2_t, w3_t = state[gi]
                scr = wpool.tile([128, GROUP], mybir.dt.bfloat16, tag="scr",
                                 name=f"scr{gi}", bufs=3)
                single = len(groups[gi]) == 1
                state[gi] = state[gi] + (scr,)
                if not single:
                    for ui in groups[gi]:
                        sig(gi, ui, 0, w1_t, scr)
                for ui in groups[gi]:
                    if unit_src[ui] != "dve":
                        continue
                    o, e = rng(gi, ui)
                    nc.vector.tensor_tensor(w2_t[:, o:e], w1_t[:, o:e],
                                            u_t[:, o:e], AluOpType.mult)
                    nc.vector.tensor_tensor(w3_t[:, o:e], w2_t[:, o:e],
                                            u2_t[:, o:e], AluOpType.mult)
                    sig(gi, ui, 1, w2_t, scr)
                    sig(gi, ui, 2, w3_t, scr)

            def emit_D(gi):
                """pool-unit sigmas (Pool has had a full extra period)."""
                _, u_t, u2_t, w1_t, w2_t, w3_t, scr = state[gi]
                for ui in groups[gi]:
                    if unit_src[ui] != "pool":
                        continue
                    sig(gi, ui, 1, w2_t, scr)
                    sig(gi, ui, 2, w3_t, scr)

            nbulk = 3 * sum(len(groups[gi]) for gi in range(G - 2))
            for gi in range(G + 3):
                if gi < G:
                    emit_A(gi)
                if 1 <= gi <= G:
                    emit_B(gi - 1)
                if 2 <= gi <= G + 1:
                    emit_C(gi - 2)
                if gi >= 3:
                    emit_D(gi - 3)
                    if gi - 3 == G - 3:
                        nc.sync.dma_start(y_dram[:, :nbulk],
                                          parts[:, :nbulk])
            nc.sync.dma_start(y_dram[:, nbulk:], parts[:, nbulk:])
    nc.finalize()
    return nc


# ---------------------------------------------------------------------------
# Driver
# ---------------------------------------------------------------------------

def _prep(rho, gamma, coords, weights):
    rho = np.asarray(rho, F32)
    gamma = np.asarray(gamma, F32)
    coords = np.asarray(coords, F32)
    weights = np.asarray(weights, F32)
    n = rho.shape[0]
    n_it = n // IT

    a, b2, f, lnf, r, norms = _derived(rho, gamma, coords, weights)
    order = _kd_order(coords, IT)
    cs, as_, b2s, lnfs, rs = (coords[order], a[order], b2[order],
                              lnf[order], r[order])
    maxargs = _survivors(cs, as_, b2s, lnfs, n_it)
    assign, slot_of, core_slots, cols, slot_sizes, units, groups = \
        _make_schedule(maxargs, n_it)
    va, ua, vb, ub = _build_vu(as_, b2s, lnfs, rs, cs)
    # stacked factors: rows [0:K_b] give Eb2, rows [0:K] give Ea + Eb2,
    # so one start=True matmul per psum region computes each exp argument
    # (no psum accumulation passes)
    vab = np.concatenate([vb, va], axis=0)
    uab = np.concatenate([ub, ua], axis=0)
    K, K_b = vab.shape[0], vb.shape[0]

    gorder = [ui for g in groups for ui in g]
    in_maps = []
    for c in range(N_CORES):
        tiles = core_slots[c]
        uc = np.concatenate(
            [uab[:, cols[tiles[units[ui][0]]][units[ui][1]:
                                              units[ui][1] + units[ui][2]]]
             for ui in gorder], axis=1)
        vc = np.concatenate(
            [vab[:, tiles[units[ui][0]] * IT:(tiles[units[ui][0]] + 1) * IT]
             for ui in gorder], axis=1)
        in_maps.append({
            "uab": np.ascontiguousarray(uc.astype(ml_dtypes.bfloat16)),
            "vab": np.ascontiguousarray(vc.astype(ml_dtypes.bfloat16)),
        })
    key = (K, K_b, tuple(units), tuple(tuple(g) for g in groups))
    meta = dict(order=order, core_slots=core_slots, norms=norms, n=n,
                gorder=gorder, units=units)
    return key, in_maps, meta


def _assemble(results, meta):
    n = meta["n"]
    norms = meta["norms"]
    order = meta["order"]
    y = np.zeros((n, 3), np.float64)
    units = meta["units"]
    for c in range(N_CORES):
        parts = np.asarray(results[c]["y"], np.float64)  # [IT, 3*NU]
        for p, ui in enumerate(meta["gorder"]):
            ib = meta["core_slots"][c][units[ui][0]]
            rows = order[ib * IT:(ib + 1) * IT]
            y[rows, :] += parts[:, 3 * p:3 * p + 3] * norms[None, :]
    return y.astype(np.float32)


def kernel_run(rho, gamma, coords, weights, **spmd_kwargs):
    from concourse.bass_utils import run_bass_kernel_spmd

    key, in_maps, meta = _prep(rho, gamma, coords, weights)
    if key not in _NC_CACHE:
        _NC_CACHE[key] = _build_nc(key)
    _LAST["key"] = key
    _LAST["meta"] = meta
    _LAST["in_maps"] = in_maps
    res = run_bass_kernel_spmd(_NC_CACHE[key], in_maps,
                               core_ids=list(range(N_CORES)), **spmd_kwargs)
    return _assemble(res.results, meta), res


def kernel(rho, gamma, coords, weights):
    y, _ = kernel_run(rho, gamma, coords, weights)
    return y
